# revision 7
# baseline (speedup 1.0000x reference)
"""MiniGPT forward pass on 8 Trainium2 NeuronCores (Bass/Tile SPMD kernel).

Model: V=32000, T=2048, D=512, H=8 heads, L=4 layers, DFF=2048, B=2, S=2048.

Sharding (8 cores, one SPMD program):
- Tokens: core c owns 512 tokens = flat[512c : 512c+512] (batch c//4).
- Attention: head-parallel within each batch group of 4 cores; core c computes
  heads (2*(c%4), 2*(c%4)+1) over its batch's full 2048 tokens. QKV and
  attention outputs are redistributed with AllToAll over the batch group.
- LM head: vocab-parallel; core c computes logits[:, 4000c:4000c+4000] for all
  4096 tokens after an AllGather of the final hidden states.

Layouts: residual h is [token, feature] fp32 in SBUF. LN outputs are cast to
bf16 and PE-transposed to [feature, token] as matmul operands. LN gamma/beta
are folded into the following matmul weights on the host.
"""
import sys

sys.path.insert(0, "/opt/trn_rl_repo")

import numpy as np
import ml_dtypes

import concourse.bass as bass
import concourse.mybir as mybir
import concourse.tile as tile
from concourse import bacc, bass_utils

BF16 = mybir.dt.bfloat16
F32 = mybir.dt.float32
I32 = mybir.dt.int32
AF = mybir.ActivationFunctionType
OP = mybir.AluOpType

V, T, D, H, L = 32000, 2048, 512, 8, 4
HD = D // H          # 64
DFF = 4 * D          # 2048
B, S = 2, 2048
NC = 8               # cores
TOK = 512            # tokens per core
VSH = V // NC        # 4000 vocab per core
NEG = -1.0e9


def build_nc():
    nc = bacc.Bacc("TRN2", target_bir_lowering=False, debug=False, num_devices=NC)

    # ---- I/O ----
    tok_emb = nc.dram_tensor("tok_emb", [V, D], F32, kind="ExternalInput")
    pos = nc.dram_tensor("pos", [TOK, D], F32, kind="ExternalInput")
    xidx = nc.dram_tensor("xidx", [TOK, 1], I32, kind="ExternalInput")
    wqkvT = nc.dram_tensor("wqkvT", [L, D, 3 * D], BF16, kind="ExternalInput")
    bqkv = nc.dram_tensor("bqkv", [L * 12 * 128, 1], F32, kind="ExternalInput")
    wprojT = nc.dram_tensor("wprojT", [L, D, D], BF16, kind="ExternalInput")
    bproj = nc.dram_tensor("bproj", [L, 1, D], F32, kind="ExternalInput")
    wffn1T = nc.dram_tensor("wffn1T", [L, D, DFF], BF16, kind="ExternalInput")
    bffn1 = nc.dram_tensor("bffn1", [L * 16 * 128, 1], F32, kind="ExternalInput")
    wffn2T = nc.dram_tensor("wffn2T", [L, DFF, D], BF16, kind="ExternalInput")
    bffn2 = nc.dram_tensor("bffn2", [L, 1, D], F32, kind="ExternalInput")
    lmT = nc.dram_tensor("lmT", [D, VSH], BF16, kind="ExternalInput")
    lmb = nc.dram_tensor("lmb", [1, VSH], F32, kind="ExternalInput")
    mstrip = nc.dram_tensor("mstrip", [128, 896], BF16, kind="ExternalInput")
    ident_in = nc.dram_tensor("ident_in", [128, 128], BF16, kind="ExternalInput")
    ones_in = nc.dram_tensor("ones_in", [1, 128], BF16, kind="ExternalInput")
    logits = nc.dram_tensor("logits", [B * S, VSH], F32, kind="ExternalOutput")

    # ---- internal DRAM (collective bounces) ----
    qkv_ai = [nc.dram_tensor(f"qkv_ai{l}", [3 * D, TOK], BF16) for l in range(L)]
    qkv_ao = [nc.dram_tensor(f"qkv_ao{l}", [3 * D, TOK], BF16) for l in range(L)]
    att_ai = [nc.dram_tensor(f"att_ai{l}", [D, TOK], BF16) for l in range(L)]
    att_ao = [nc.dram_tensor(f"att_ao{l}", [D, TOK], BF16) for l in range(L)]
    ag_in = nc.dram_tensor("ag_in", [D, TOK], BF16)
    ag_out = nc.dram_tensor("ag_out", [NC * D, TOK], BF16, addr_space="Shared")
    grp = [list(range(NC))]

    with tile.TileContext(nc) as tc:
        with (
            tc.tile_pool(name="const", bufs=1) as cp,
            tc.tile_pool(name="persist", bufs=1) as pp,
        ):
            ident = cp.tile([128, 128], BF16, name="ident")
            ones_r = cp.tile([1, 128], BF16, name="ones_r")
            msk = cp.tile([128, 896], BF16, name="msk")
            projb_bc = cp.tile([128, L * D], BF16, name="projb_bc")
            ffn2b_bc = cp.tile([128, L * D], BF16, name="ffn2b_bc")
            lmb_bc = cp.tile([128, VSH], BF16, name="lmb_bc")
            brow = cp.tile([1, 512], F32, name="brow")
            brow_bf = cp.tile([1, 512], BF16, name="brow_bf")
            eps_t = cp.tile([128, 1], F32, name="eps_t")
            h = pp.tile([128, 4 * D], F32, name="h")
            idx_sb = pp.tile([128, 4], I32, name="idx_sb")
            vones = pp.tile([128, 32 * 65], BF16, name="vones")

            with (
                tc.tile_pool(name="wpool", bufs=1) as wp,
                tc.tile_pool(name="work", bufs=2) as wk,
                tc.tile_pool(name="exppool", bufs=3) as ep,
                tc.tile_pool(name="pmm", bufs=2, space="PSUM") as pmm,
                tc.tile_pool(name="psc", bufs=2, space="PSUM") as psc,
                tc.tile_pool(name="pout", bufs=1, space="PSUM") as pout,
            ):
                # ================= prologue =================
                nc.sync.dma_start(out=ident[:], in_=ident_in[:])
                nc.sync.dma_start(out=ones_r[:], in_=ones_in[:])
                nc.sync.dma_start(out=msk[:], in_=mstrip[:])
                nc.vector.memset(eps_t[:], 1e-5)

                def bcast_row(dst_ap, src_dram_ap, n):
                    # dst[128, n] = broadcast of src[1, n] across partitions
                    done = 0
                    while done < n:
                        w = min(512, n - done)
                        nc.sync.dma_start(out=brow[:, :w], in_=src_dram_ap[:, done:done + w])
                        nc.vector.tensor_copy(out=brow_bf[:, :w], in_=brow[:, :w])
                        ps = pmm.tile([128, 512], F32, tag="pmm")
                        nc.tensor.matmul(ps[:, :w], lhsT=ones_r[:, :], rhs=brow_bf[:, :w],
                                         start=True, stop=True)
                        nc.vector.tensor_copy(out=dst_ap[:, done:done + w], in_=ps[:, :w])
                        done += w

                for l in range(L):
                    bcast_row(projb_bc[:, l * D:(l + 1) * D], bproj[l], D)
                    bcast_row(ffn2b_bc[:, l * D:(l + 1) * D], bffn2[l], D)
                bcast_row(lmb_bc[:, :], lmb[:, :], VSH)

                # embeddings -> residual h [128 tok, 4 blocks * 512 feat] fp32
                for t in range(4):
                    nc.sync.dma_start(out=idx_sb[:, t:t + 1], in_=xidx[128 * t:128 * (t + 1), :])
                for t in range(4):
                    emb = wk.tile([128, D], F32, tag="emb", bufs=1)
                    nc.gpsimd.indirect_dma_start(
                        out=emb[:], out_offset=None, in_=tok_emb[:],
                        in_offset=bass.IndirectOffsetOnAxis(ap=idx_sb[:, t:t + 1], axis=0),
                    )
                    pos_t = wk.tile([128, D], F32, tag="emb2", bufs=1)
                    nc.sync.dma_start(out=pos_t[:], in_=pos[128 * t:128 * (t + 1), :])
                    nc.vector.tensor_tensor(out=h[:, D * t:D * (t + 1)], in0=emb[:], in1=pos_t[:], op=OP.add)

                nc.vector.memset(vones[:], 1.0)

                # ---- helpers ----
                def layernorm_T(src_ap, dst_bf_T, eps=1e-5):
                    """src_ap: [128, 4*D] fp32 ([tok, feat] blocks). Writes dst_bf_T
                    [128, 4*512] bf16 = transposed ([feat-ptile, tok]) normalized."""
                    hln = wk.tile([128, 4 * D], BF16, tag="hln", bufs=1)
                    for t in range(4):
                        s = wk.tile([128, 1], F32, tag="lns")
                        ssq = wk.tile([128, 1], F32, tag="lns")
                        sq = wk.tile([128, D], F32, tag="lnsq", bufs=1)
                        blk = src_ap[:, D * t:D * (t + 1)]
                        nc.vector.tensor_reduce(out=s[:], in_=blk, axis=mybir.AxisListType.X, op=OP.add)
                        nc.vector.tensor_tensor(out=sq[:], in0=blk, in1=blk, op=OP.mult)
                        nc.vector.tensor_reduce(out=ssq[:], in_=sq[:], axis=mybir.AxisListType.X, op=OP.add)
                        nmu = wk.tile([128, 1], F32, tag="lnmu")
                        nc.vector.tensor_scalar_mul(nmu[:], s[:], -1.0 / D)
                        var = wk.tile([128, 1], F32, tag="lns")
                        nc.vector.tensor_tensor(out=var[:], in0=nmu[:], in1=nmu[:], op=OP.mult)
                        nc.vector.scalar_tensor_tensor(out=var[:], in0=ssq[:], scalar=1.0 / D,
                                                       in1=var[:], op0=OP.mult, op1=OP.subtract)
                        rs = wk.tile([128, 1], F32, tag="lnrs")
                        nc.scalar.activation(out=rs[:], in_=var[:], func=AF.Ln, bias=eps_t[:])
                        nc.scalar.activation(out=rs[:], in_=rs[:], func=AF.Exp, scale=-0.5)
                        nc.vector.tensor_scalar(out=hln[:, D * t:D * (t + 1)], in0=blk,
                                                scalar1=nmu[:], scalar2=rs[:], op0=OP.add, op1=OP.mult)
                    for f in range(4):
                        tp = pmm.tile([128, 512], BF16, tag="pmm")
                        for t in range(4):
                            nc.tensor.transpose(out=tp[:, 128 * t:128 * (t + 1)],
                                                in_=hln[:, D * t + 128 * f: D * t + 128 * (f + 1)],
                                                identity=ident[:])
                        nc.vector.tensor_copy(out=dst_bf_T[:, 512 * f:512 * (f + 1)], in_=tp[:])

                # ================= transformer layers =================
                for l in range(L):
                    wq = wp.tile([128, 4 * 1536], BF16, tag="wq")
                    nc.sync.dma_start(out=wq[:].rearrange("p (c e) -> p c e", c=4),
                                      in_=wqkvT[l].rearrange("(c p) e -> p c e", p=128))
                    wpj = wp.tile([128, 4 * 512], BF16, tag="wpj")
                    nc.sync.dma_start(out=wpj[:].rearrange("p (c e) -> p c e", c=4),
                                      in_=wprojT[l].rearrange("(c p) e -> p c e", p=128))
                    wf1 = wp.tile([128, 4 * 2048], BF16, tag="wf1")
                    nc.sync.dma_start(out=wf1[:].rearrange("p (c e) -> p c e", c=4),
                                      in_=wffn1T[l].rearrange("(c p) e -> p c e", p=128))
                    wf2 = wp.tile([128, 16 * 512], BF16, tag="wf2")
                    nc.sync.dma_start(out=wf2[:].rearrange("p (c e) -> p c e", c=16),
                                      in_=wffn2T[l].rearrange("(c p) e -> p c e", p=128))
                    bq = wp.tile([128, 12], F32, tag="bq")
                    for o in range(12):
                        nc.sync.dma_start(out=bq[:, o:o + 1],
                                          in_=bqkv[(l * 12 + o) * 128:(l * 12 + o + 1) * 128, :])
                    bf1 = wp.tile([128, 16], F32, tag="bf1")
                    for o in range(16):
                        nc.sync.dma_start(out=bf1[:, o:o + 1],
                                          in_=bffn1[(l * 16 + o) * 128:(l * 16 + o + 1) * 128, :])

                    # -- LN1 + transpose --
                    hlnT = wk.tile([128, 4 * 512], BF16, tag="hlnT", bufs=1)
                    layernorm_T(h[:], hlnT)

                    # -- qkvT = W' @ hlnT  ([3D feat, 512 tok]) --
                    qkvT = wk.tile([128, 12 * 512], BF16, tag="qkvT", bufs=1)
                    for o in range(12):
                        ps = pmm.tile([128, 512], F32, tag="pmm")
                        for kc in range(4):
                            nc.tensor.matmul(ps[:],
                                             lhsT=wq[:, 1536 * kc + 128 * o:1536 * kc + 128 * (o + 1)],
                                             rhs=hlnT[:, 512 * kc:512 * (kc + 1)],
                                             start=(kc == 0), stop=(kc == 3))
                        nc.vector.tensor_scalar(out=qkvT[:, 512 * o:512 * (o + 1)], in0=ps[:],
                                                scalar1=bq[:, o:o + 1], scalar2=None, op0=OP.add)
                    # A2A over 8 cores: shard s rows [192s,+192) = head-s q/k/v of my tokens
                    for s_ in range(8):
                        pb = 64 * (s_ % 2)
                        blk = s_ // 2
                        nc.sync.dma_start(out=qkv_ai[l][192 * s_:192 * s_ + 64, :],
                                          in_=qkvT[pb:pb + 64, 512 * blk:512 * (blk + 1)])
                        nc.sync.dma_start(out=qkv_ai[l][192 * s_ + 64:192 * s_ + 128, :],
                                          in_=qkvT[pb:pb + 64, 512 * (4 + blk):512 * (5 + blk)])
                        nc.sync.dma_start(out=qkv_ai[l][192 * s_ + 128:192 * s_ + 192, :],
                                          in_=qkvT[pb:pb + 64, 512 * (8 + blk):512 * (9 + blk)])
                    nc.gpsimd.collective_compute(
                        "AllToAll", OP.bypass, replica_groups=grp,
                        ins=[qkv_ai[l][:]], outs=[qkv_ao[l][:]],
                    )
                    # my head over both batches: batch b in partition half 64b
                    qT = wk.tile([128, 2048], BF16, tag="qT", bufs=1)
                    kT = wk.tile([128, 2048], BF16, tag="kT", bufs=1)
                    vT = wk.tile([128, 2048], BF16, tag="vT", bufs=1)
                    for r in range(8):
                        b_, rr = r // 4, r % 4
                        nc.sync.dma_start(out=qT[64 * b_:64 * b_ + 64, 512 * rr:512 * (rr + 1)],
                                          in_=qkv_ao[l][192 * r:192 * r + 64, :])
                        nc.sync.dma_start(out=kT[64 * b_:64 * b_ + 64, 512 * rr:512 * (rr + 1)],
                                          in_=qkv_ao[l][192 * r + 64:192 * r + 128, :])
                        nc.sync.dma_start(out=vT[64 * b_:64 * b_ + 64, 512 * rr:512 * (rr + 1)],
                                          in_=qkv_ao[l][192 * r + 128:192 * r + 192, :])
                    for b_ in range(2):
                        hb = 64 * b_
                        for i in range(16):
                            tp = pmm.tile([128, 64], BF16, tag="pmm")
                            nc.tensor.transpose(out=tp[:], in_=vT[hb:hb + 64, 128 * i:128 * (i + 1)],
                                                identity=ident[hb:hb + 64, hb:hb + 64])
                            nc.vector.tensor_copy(out=vones[:, 65 * (16 * b_ + i):65 * (16 * b_ + i) + 64],
                                                  in_=tp[:])

                    # -- attention (my head, both batches, causal, q in 1024-pairs) --
                    attnT = wk.tile([64, 4096], BF16, tag="attnT", bufs=1)
                    for b_ in range(2):
                        hb = 64 * b_
                        for p in range(2):
                            outp = pout.tile([65, 1024], F32, tag="pout")
                            for i in range(8 * p + 8):
                                jlmin = max(0, i // 4 - 2 * p)
                                sc = psc.tile([128, 1024], F32, tag="psc")
                                for jl in (0, 1):
                                    if jl < jlmin:
                                        continue
                                    nc.tensor.matmul(
                                        sc[:, 512 * jl:512 * (jl + 1)],
                                        lhsT=kT[hb:hb + 64, 128 * i:128 * (i + 1)],
                                        rhs=qT[hb:hb + 64, 1024 * p + 512 * jl:1024 * p + 512 * (jl + 1)],
                                        start=True, stop=True)
                                    if i // 4 == 2 * p + jl:
                                        m = i % 4
                                        nc.vector.tensor_tensor(
                                            out=sc[:, 512 * jl:512 * (jl + 1)],
                                            in0=sc[:, 512 * jl:512 * (jl + 1)],
                                            in1=msk[:, 384 - 128 * m:896 - 128 * m], op=OP.add)
                                ex = ep.tile([128, 1024], BF16, tag="ex")
                                nc.scalar.activation(out=ex[:, 512 * jlmin:1024],
                                                     in_=sc[:, 512 * jlmin:1024],
                                                     func=AF.Exp, scale=float(HD) ** -0.5)
                                for jl in (0, 1):
                                    if jl < jlmin:
                                        continue
                                    kmax = 4 * (2 * p + jl) + 3
                                    if i > kmax:
                                        continue
                                    nc.tensor.matmul(
                                        outp[:, 512 * jl:512 * (jl + 1)],
                                        lhsT=vones[:, 65 * (16 * b_ + i):65 * (16 * b_ + i + 1)],
                                        rhs=ex[:, 512 * jl:512 * (jl + 1)],
                                        start=(i == 0), stop=(i == kmax))
                            # normalize: rows 0..63 /= row 64
                            rc = wk.tile([1, 1024], F32, tag="rc", bufs=1)
                            nc.vector.reciprocal(out=rc[:], in_=outp[64:65, :])
                            rcb = wk.tile([1, 1024], BF16, tag="rcb", bufs=1)
                            nc.vector.tensor_copy(out=rcb[:], in_=rc[:])
                            bc = psc.tile([128, 1024], F32, tag="psc")
                            for q2 in range(2):
                                nc.tensor.matmul(bc[0:64, 512 * q2:512 * (q2 + 1)], lhsT=ones_r[:, 0:64],
                                                 rhs=rcb[:, 512 * q2:512 * (q2 + 1)], start=True, stop=True)
                            bcs = wk.tile([64, 1024], BF16, tag="bcs", bufs=1)
                            nc.vector.tensor_copy(out=bcs[:], in_=bc[0:64, :])
                            nc.vector.tensor_tensor(out=attnT[:, 2048 * b_ + 1024 * p:2048 * b_ + 1024 * (p + 1)],
                                                    in0=outp[0:64, :], in1=bcs[:], op=OP.mult)
                    # A2A attention outputs back to token owners
                    for s_ in range(8):
                        nc.sync.dma_start(out=att_ai[l][64 * s_:64 * (s_ + 1), :],
                                          in_=attnT[:, 512 * s_:512 * (s_ + 1)])
                    nc.gpsimd.collective_compute(
                        "AllToAll", OP.bypass, replica_groups=grp,
                        ins=[att_ai[l][:]], outs=[att_ao[l][:]],
                    )
                    aT = wk.tile([128, 4 * 512], BF16, tag="aT", bufs=1)
                    for r in range(4):
                        nc.sync.dma_start(out=aT[:, 512 * r:512 * (r + 1)],
                                          in_=att_ao[l][128 * r:128 * (r + 1), :])

                    # -- proj + residual --
                    for t in range(4):
                        ps = pmm.tile([128, 512], F32, tag="pmm")
                        for fc in range(4):
                            nc.tensor.matmul(ps[:],
                                             lhsT=aT[:, 512 * fc + 128 * t:512 * fc + 128 * (t + 1)],
                                             rhs=wpj[:, 512 * fc:512 * (fc + 1)],
                                             start=(fc == 0), stop=(fc == 3))
                        nc.vector.tensor_tensor(out=ps[:], in0=ps[:],
                                                in1=projb_bc[:, D * l:D * (l + 1)], op=OP.add)
                        nc.vector.tensor_tensor(out=h[:, D * t:D * (t + 1)],
                                                in0=h[:, D * t:D * (t + 1)], in1=ps[:], op=OP.add)

                    # -- LN2 + FFN --
                    hln2T = wk.tile([128, 4 * 512], BF16, tag="hlnT", bufs=1)
                    layernorm_T(h[:], hln2T)
                    fT = wk.tile([128, 16 * 512], BF16, tag="fT", bufs=1)
                    for o in range(16):
                        ps = pmm.tile([128, 512], F32, tag="pmm")
                        for kc in range(4):
                            nc.tensor.matmul(ps[:],
                                             lhsT=wf1[:, 2048 * kc + 128 * o:2048 * kc + 128 * (o + 1)],
                                             rhs=hln2T[:, 512 * kc:512 * (kc + 1)],
                                             start=(kc == 0), stop=(kc == 3))
                        nc.scalar.activation(out=fT[:, 512 * o:512 * (o + 1)], in_=ps[:],
                                             func=AF.Gelu, bias=bf1[:, o:o + 1])
                    for t in range(4):
                        ps = pmm.tile([128, 512], F32, tag="pmm")
                        for kc in range(16):
                            nc.tensor.matmul(ps[:],
                                             lhsT=fT[:, 512 * kc + 128 * t:512 * kc + 128 * (t + 1)],
                                             rhs=wf2[:, 512 * kc:512 * (kc + 1)],
                                             start=(kc == 0), stop=(kc == 15))
                        nc.vector.tensor_tensor(out=ps[:], in0=ps[:],
                                                in1=ffn2b_bc[:, D * l:D * (l + 1)], op=OP.add)
                        nc.vector.tensor_tensor(out=h[:, D * t:D * (t + 1)],
                                                in0=h[:, D * t:D * (t + 1)], in1=ps[:], op=OP.add)

                # ================= final LN + AllGather =================
                hfT = wk.tile([128, 4 * 512], BF16, tag="hlnT", bufs=1)
                layernorm_T(h[:], hfT)
                for f in range(4):
                    nc.sync.dma_start(out=ag_in[128 * f:128 * (f + 1), :],
                                      in_=hfT[:, 512 * f:512 * (f + 1)])
                nc.gpsimd.collective_compute(
                    "AllGather", OP.bypass, replica_groups=[list(range(NC))],
                    ins=[ag_in[:]], outs=[ag_out[:]],
                )

            # ================= LM head (body pools closed, PSUM free) ========
            with (
                tc.tile_pool(name="lmw", bufs=1) as lw,
                tc.tile_pool(name="lmwork", bufs=3) as lk,
                tc.tile_pool(name="plm", bufs=2, space="PSUM") as plm,
            ):
                lmw = lw.tile([128, 4 * VSH], BF16, name="lmw")
                nc.sync.dma_start(out=lmw[:].rearrange("p (c e) -> p c e", c=4),
                                  in_=lmT[:].rearrange("(c p) e -> p c e", p=128))
                for tt in range(32):
                    r, u = tt // 4, tt % 4
                    lhs = lk.tile([128, 4 * 128], BF16, tag="lhs")
                    for kc in range(4):
                        nc.sync.dma_start(out=lhs[:, 128 * kc:128 * (kc + 1)],
                                          in_=ag_out[512 * r + 128 * kc:512 * r + 128 * (kc + 1),
                                                     128 * u:128 * (u + 1)])
                    stage = lk.tile([128, VSH], F32, tag="stage")
                    for hf in range(2):
                        ps = plm.tile([128, 2048], F32, tag="plm")
                        for kc in range(4):
                            for vc in range(4):
                                w0 = VSH * kc + 2000 * hf + 500 * vc
                                nc.tensor.matmul(
                                    ps[:, 512 * vc:512 * vc + 500],
                                    lhsT=lhs[:, 128 * kc:128 * (kc + 1)],
                                    rhs=lmw[:, w0:w0 + 500],
                                    start=(kc == 0), stop=(kc == 3))
                        ps3 = ps[:].rearrange("p (b e) -> p b e", b=4)[:, :, 0:500]
                        st3 = stage[:, 2000 * hf:2000 * (hf + 1)].rearrange("p (b e) -> p b e", b=4)
                        bc3 = lmb_bc[:, 2000 * hf:2000 * (hf + 1)].rearrange("p (b e) -> p b e", b=4)
                        nc.vector.tensor_tensor(out=st3, in0=ps3, in1=bc3, op=OP.add)
                    nc.sync.dma_start(out=logits[128 * tt:128 * (tt + 1), :], in_=stage[:])

    nc.compile()
    return nc


_NC_CACHE = None


def _get_nc():
    global _NC_CACHE
    if _NC_CACHE is None:
        _NC_CACHE = build_nc()
    return _NC_CACHE


def _prep_inputs(inputs):
    bf = ml_dtypes.bfloat16
    tok_emb = np.asarray(inputs["tok_emb"], np.float32)
    pos_emb = np.asarray(inputs["pos_emb"], np.float32)
    x = np.asarray(inputs["x"]).astype(np.int32).reshape(-1)  # [4096] flat

    def eff(w, g, b, wb):
        # fold the preceding layernorm's gamma/beta into w (out,in) and bias
        w = np.asarray(w, np.float32)
        weff = w * np.asarray(g, np.float32)[None, :]
        beff = w @ np.asarray(b, np.float32) + np.asarray(wb, np.float32)
        return weff, beff

    wqkvT = np.zeros((L, D, 3 * D), bf)
    bqkv = np.zeros((L, 12, 128), np.float32)
    wprojT = np.zeros((L, D, D), bf)
    bproj = np.zeros((L, 1, D), np.float32)
    wffn1T = np.zeros((L, D, DFF), bf)
    bffn1 = np.zeros((L, 16, 128), np.float32)
    wffn2T = np.zeros((L, DFF, D), bf)
    bffn2 = np.zeros((L, 1, D), np.float32)
    for l in range(L):
        w, b = eff(inputs["qkv_w"][l], inputs["ln1_g"][l], inputs["ln1_b"][l], inputs["qkv_b"][l])
        wqkvT[l] = w.T.astype(bf)
        bqkv[l] = b.reshape(12, 128)
        wprojT[l] = np.asarray(inputs["proj_w"][l], np.float32).T.astype(bf)
        bproj[l, 0] = np.asarray(inputs["proj_b"][l], np.float32)
        w, b = eff(inputs["ffn1_w"][l], inputs["ln2_g"][l], inputs["ln2_b"][l], inputs["ffn1_b"][l])
        wffn1T[l] = w.T.astype(bf)
        bffn1[l] = b.reshape(16, 128)
        wffn2T[l] = np.asarray(inputs["ffn2_w"][l], np.float32).T.astype(bf)
        bffn2[l, 0] = np.asarray(inputs["ffn2_b"][l], np.float32)
    lmw, lmbf = eff(inputs["lm_w"], inputs["lnf_g"], inputs["lnf_b"], inputs["lm_b"])

    mstrip = np.full((128, 896), NEG, np.float32)
    kk = np.arange(128)[:, None]
    cc = np.arange(896)[None, :]
    mstrip[kk <= cc - 384] = 0.0
    mstrip = mstrip.astype(bf)

    common = dict(tok_emb=tok_emb, wqkvT=wqkvT, bqkv=bqkv.reshape(L * 12 * 128, 1),
                  wprojT=wprojT, bproj=bproj, wffn1T=wffn1T,
                  bffn1=bffn1.reshape(L * 16 * 128, 1), wffn2T=wffn2T, bffn2=bffn2,
                  mstrip=mstrip, ident_in=np.eye(128, dtype=bf),
                  ones_in=np.ones((1, 128), bf))
    in_maps = []
    for c in range(NC):
        s0 = 512 * (c % 4)
        m = dict(common)
        m["pos"] = pos_emb[s0:s0 + 512]
        m["xidx"] = x[512 * c:512 * (c + 1)].reshape(TOK, 1)
        m["lmT"] = np.ascontiguousarray(lmw[VSH * c:VSH * (c + 1)].T.astype(bf))
        m["lmb"] = lmbf[VSH * c:VSH * (c + 1)].reshape(1, VSH).copy()
        in_maps.append(m)
    return in_maps


def run(inputs, trace=False, tmpdir=None):
    nc = _get_nc()
    in_maps = _prep_inputs(inputs)
    res = bass_utils.run_bass_kernel_spmd(nc, in_maps, list(range(NC)), trace=trace, tmpdir=tmpdir)
    full = np.empty((B * S, V), np.float32)
    for c in range(NC):
        full[:, VSH * c:VSH * (c + 1)] = res.results[c]["logits"]
    return full.reshape(B, S, V), res


def kernel(**inputs) -> np.ndarray:
    out, _ = run(inputs)
    return out


# revision 9
# speedup vs baseline: 1.0415x; 1.0415x over previous
"""MiniGPT forward pass on 8 Trainium2 NeuronCores (Bass/Tile SPMD kernel).

Model: V=32000, T=2048, D=512, H=8 heads, L=4 layers, DFF=2048, B=2, S=2048.

Sharding (8 cores, one SPMD program):
- Tokens: core c owns 512 tokens = flat[512c : 512c+512] (batch c//4).
- Attention: head-parallel within each batch group of 4 cores; core c computes
  heads (2*(c%4), 2*(c%4)+1) over its batch's full 2048 tokens. QKV and
  attention outputs are redistributed with AllToAll over the batch group.
- LM head: vocab-parallel; core c computes logits[:, 4000c:4000c+4000] for all
  4096 tokens after an AllGather of the final hidden states.

Layouts: residual h is [token, feature] fp32 in SBUF. LN outputs are cast to
bf16 and PE-transposed to [feature, token] as matmul operands. LN gamma/beta
are folded into the following matmul weights on the host.
"""
import sys

sys.path.insert(0, "/opt/trn_rl_repo")

import numpy as np
import ml_dtypes

import concourse.bass as bass
import concourse.mybir as mybir
import concourse.tile as tile
from concourse import bacc, bass_utils

BF16 = mybir.dt.bfloat16
F32 = mybir.dt.float32
I32 = mybir.dt.int32
AF = mybir.ActivationFunctionType
OP = mybir.AluOpType

V, T, D, H, L = 32000, 2048, 512, 8, 4
HD = D // H          # 64
DFF = 4 * D          # 2048
B, S = 2, 2048
NC = 8               # cores
TOK = 512            # tokens per core
VSH = V // NC        # 4000 vocab per core
NEG = -1.0e9


def build_nc():
    nc = bacc.Bacc("TRN2", target_bir_lowering=False, debug=False, num_devices=NC)

    # ---- I/O ----
    tok_emb = nc.dram_tensor("tok_emb", [V, D], F32, kind="ExternalInput")
    pos = nc.dram_tensor("pos", [TOK, D], F32, kind="ExternalInput")
    xidx = nc.dram_tensor("xidx", [TOK, 1], I32, kind="ExternalInput")
    wqkvT = nc.dram_tensor("wqkvT", [L, D, 3 * D], BF16, kind="ExternalInput")
    bqkv = nc.dram_tensor("bqkv", [L * 12 * 128, 1], F32, kind="ExternalInput")
    wprojT = nc.dram_tensor("wprojT", [L, D, D], BF16, kind="ExternalInput")
    bproj = nc.dram_tensor("bproj", [L, 1, D], F32, kind="ExternalInput")
    wffn1T = nc.dram_tensor("wffn1T", [L, D, DFF], BF16, kind="ExternalInput")
    bffn1 = nc.dram_tensor("bffn1", [L * 16 * 128, 1], F32, kind="ExternalInput")
    wffn2T = nc.dram_tensor("wffn2T", [L, DFF, D], BF16, kind="ExternalInput")
    bffn2 = nc.dram_tensor("bffn2", [L, 1, D], F32, kind="ExternalInput")
    lmT = nc.dram_tensor("lmT", [D, VSH], BF16, kind="ExternalInput")
    lmb = nc.dram_tensor("lmb", [1, VSH], F32, kind="ExternalInput")
    mstrip = nc.dram_tensor("mstrip", [128, 896], BF16, kind="ExternalInput")
    ident_in = nc.dram_tensor("ident_in", [128, 128], BF16, kind="ExternalInput")
    ones_in = nc.dram_tensor("ones_in", [1, 128], BF16, kind="ExternalInput")
    logits = nc.dram_tensor("logits", [B * S, VSH], F32, kind="ExternalOutput")

    # ---- internal DRAM (collective bounces) ----
    qkv_ai = [nc.dram_tensor(f"qkv_ai{l}", [3 * D, TOK], BF16) for l in range(L)]
    qkv_ao = [nc.dram_tensor(f"qkv_ao{l}", [3 * D, TOK], BF16) for l in range(L)]
    att_ai = [nc.dram_tensor(f"att_ai{l}", [D, TOK], BF16) for l in range(L)]
    att_ao = [nc.dram_tensor(f"att_ao{l}", [D, TOK], BF16) for l in range(L)]
    ag_in = nc.dram_tensor("ag_in", [D, TOK], BF16)
    ag_out = nc.dram_tensor("ag_out", [NC * D, TOK], BF16, addr_space="Shared")
    grp = [list(range(NC))]

    with tile.TileContext(nc) as tc:
        with (
            tc.tile_pool(name="const", bufs=1) as cp,
            tc.tile_pool(name="persist", bufs=1) as pp,
        ):
            ident = cp.tile([128, 128], BF16, name="ident")
            ones_r = cp.tile([1, 128], BF16, name="ones_r")
            msk = cp.tile([128, 896], BF16, name="msk")
            projb_bc = cp.tile([128, L * D], BF16, name="projb_bc")
            ffn2b_bc = cp.tile([128, L * D], BF16, name="ffn2b_bc")
            lmb_bc = cp.tile([128, VSH], BF16, name="lmb_bc")
            brow = cp.tile([1, 512], F32, name="brow")
            brow_bf = cp.tile([1, 512], BF16, name="brow_bf")
            eps_t = cp.tile([128, 1], F32, name="eps_t")
            h = pp.tile([128, 4 * D], F32, name="h")
            idx_sb = pp.tile([128, 4], I32, name="idx_sb")
            vones = pp.tile([128, 32 * 65], BF16, name="vones")

            with (
                tc.tile_pool(name="wpool", bufs=1) as wp,
                tc.tile_pool(name="work", bufs=2) as wk,
                tc.tile_pool(name="exppool", bufs=3) as ep,
                tc.tile_pool(name="pmm", bufs=2, space="PSUM") as pmm,
                tc.tile_pool(name="psc", bufs=2, space="PSUM") as psc,
                tc.tile_pool(name="pout", bufs=2, space="PSUM") as pout,
            ):
                # ================= prologue =================
                nc.sync.dma_start(out=ident[:], in_=ident_in[:])
                nc.sync.dma_start(out=ones_r[:], in_=ones_in[:])
                nc.sync.dma_start(out=msk[:], in_=mstrip[:])
                nc.vector.memset(eps_t[:], 1e-5)

                def bcast_row(dst_ap, src_dram_ap, n):
                    # dst[128, n] = broadcast of src[1, n] across partitions
                    done = 0
                    while done < n:
                        w = min(512, n - done)
                        nc.sync.dma_start(out=brow[:, :w], in_=src_dram_ap[:, done:done + w])
                        nc.vector.tensor_copy(out=brow_bf[:, :w], in_=brow[:, :w])
                        ps = pmm.tile([128, 512], F32, tag="pmm")
                        nc.tensor.matmul(ps[:, :w], lhsT=ones_r[:, :], rhs=brow_bf[:, :w],
                                         start=True, stop=True)
                        nc.vector.tensor_copy(out=dst_ap[:, done:done + w], in_=ps[:, :w])
                        done += w

                for l in range(L):
                    bcast_row(projb_bc[:, l * D:(l + 1) * D], bproj[l], D)
                    bcast_row(ffn2b_bc[:, l * D:(l + 1) * D], bffn2[l], D)
                bcast_row(lmb_bc[:, :], lmb[:, :], VSH)

                # embeddings -> residual h [128 tok, 4 blocks * 512 feat] fp32
                for t in range(4):
                    nc.sync.dma_start(out=idx_sb[:, t:t + 1], in_=xidx[128 * t:128 * (t + 1), :])
                for t in range(4):
                    emb = wk.tile([128, D], F32, tag="emb", bufs=1)
                    nc.gpsimd.indirect_dma_start(
                        out=emb[:], out_offset=None, in_=tok_emb[:],
                        in_offset=bass.IndirectOffsetOnAxis(ap=idx_sb[:, t:t + 1], axis=0),
                    )
                    pos_t = wk.tile([128, D], F32, tag="emb2", bufs=1)
                    nc.sync.dma_start(out=pos_t[:], in_=pos[128 * t:128 * (t + 1), :])
                    nc.vector.tensor_tensor(out=h[:, D * t:D * (t + 1)], in0=emb[:], in1=pos_t[:], op=OP.add)

                nc.vector.memset(vones[:], 1.0)

                # ---- helpers ----
                def layernorm_T(src_ap, dst_bf_T, eps=1e-5):
                    """src_ap: [128, 4*D] fp32 ([tok, feat] blocks). Writes dst_bf_T
                    [128, 4*512] bf16 = transposed ([feat-ptile, tok]) normalized."""
                    hln = wk.tile([128, 4 * D], BF16, tag="hln", bufs=1)
                    nmu4 = wk.tile([128, 4], F32, tag="lnmu")
                    var4 = wk.tile([128, 4], F32, tag="lnvar")
                    rs4 = wk.tile([128, 4], F32, tag="lnrs")
                    for t in range(4):
                        s = wk.tile([128, 1], F32, tag="lns")
                        ssq = wk.tile([128, 1], F32, tag="lns")
                        sq = wk.tile([128, D], F32, tag="lnsq", bufs=1)
                        blk = src_ap[:, D * t:D * (t + 1)]
                        nc.vector.tensor_reduce(out=s[:], in_=blk, axis=mybir.AxisListType.X, op=OP.add)
                        nc.vector.tensor_tensor(out=sq[:], in0=blk, in1=blk, op=OP.mult)
                        nc.vector.tensor_reduce(out=ssq[:], in_=sq[:], axis=mybir.AxisListType.X, op=OP.add)
                        nc.vector.tensor_scalar_mul(nmu4[:, t:t + 1], s[:], -1.0 / D)
                        mu2 = wk.tile([128, 1], F32, tag="lns")
                        nc.vector.tensor_tensor(out=mu2[:], in0=nmu4[:, t:t + 1], in1=nmu4[:, t:t + 1], op=OP.mult)
                        nc.vector.scalar_tensor_tensor(out=var4[:, t:t + 1], in0=ssq[:], scalar=1.0 / D,
                                                       in1=mu2[:], op0=OP.mult, op1=OP.subtract)
                    nc.scalar.activation(out=rs4[:], in_=var4[:], func=AF.Ln, bias=eps_t[:])
                    nc.scalar.activation(out=rs4[:], in_=rs4[:], func=AF.Exp, scale=-0.5)
                    for t in range(4):
                        nc.vector.tensor_scalar(out=hln[:, D * t:D * (t + 1)], in0=src_ap[:, D * t:D * (t + 1)],
                                                scalar1=nmu4[:, t:t + 1], scalar2=rs4[:, t:t + 1],
                                                op0=OP.add, op1=OP.mult)
                    for f in range(4):
                        tp = pmm.tile([128, 512], BF16, tag="pmm")
                        for t in range(4):
                            nc.tensor.transpose(out=tp[:, 128 * t:128 * (t + 1)],
                                                in_=hln[:, D * t + 128 * f: D * t + 128 * (f + 1)],
                                                identity=ident[:])
                        nc.vector.tensor_copy(out=dst_bf_T[:, 512 * f:512 * (f + 1)], in_=tp[:])

                # ================= transformer layers =================
                for l in range(L):
                    wq = wp.tile([128, 4 * 1536], BF16, tag="wq")
                    nc.sync.dma_start(out=wq[:].rearrange("p (c e) -> p c e", c=4),
                                      in_=wqkvT[l].rearrange("(c p) e -> p c e", p=128))
                    wpj = wp.tile([128, 4 * 512], BF16, tag="wpj")
                    nc.sync.dma_start(out=wpj[:].rearrange("p (c e) -> p c e", c=4),
                                      in_=wprojT[l].rearrange("(c p) e -> p c e", p=128))
                    wf1 = wp.tile([128, 4 * 2048], BF16, tag="wf1")
                    nc.sync.dma_start(out=wf1[:].rearrange("p (c e) -> p c e", c=4),
                                      in_=wffn1T[l].rearrange("(c p) e -> p c e", p=128))
                    wf2 = wp.tile([128, 16 * 512], BF16, tag="wf2")
                    nc.sync.dma_start(out=wf2[:].rearrange("p (c e) -> p c e", c=16),
                                      in_=wffn2T[l].rearrange("(c p) e -> p c e", p=128))
                    bq = wp.tile([128, 12], F32, tag="bq")
                    for o in range(12):
                        nc.sync.dma_start(out=bq[:, o:o + 1],
                                          in_=bqkv[(l * 12 + o) * 128:(l * 12 + o + 1) * 128, :])
                    bf1 = wp.tile([128, 16], F32, tag="bf1")
                    for o in range(16):
                        nc.sync.dma_start(out=bf1[:, o:o + 1],
                                          in_=bffn1[(l * 16 + o) * 128:(l * 16 + o + 1) * 128, :])

                    # -- LN1 + transpose --
                    hlnT = wk.tile([128, 4 * 512], BF16, tag="hlnT", bufs=1)
                    layernorm_T(h[:], hlnT)

                    # -- qkvT = W' @ hlnT  ([3D feat, 512 tok]) --
                    qkvT = wk.tile([128, 12 * 512], BF16, tag="qkvT", bufs=1)
                    for o in range(12):
                        ps = pmm.tile([128, 512], F32, tag="pmm")
                        for kc in range(4):
                            nc.tensor.matmul(ps[:],
                                             lhsT=wq[:, 1536 * kc + 128 * o:1536 * kc + 128 * (o + 1)],
                                             rhs=hlnT[:, 512 * kc:512 * (kc + 1)],
                                             start=(kc == 0), stop=(kc == 3))
                        nc.vector.tensor_scalar(out=qkvT[:, 512 * o:512 * (o + 1)], in0=ps[:],
                                                scalar1=bq[:, o:o + 1], scalar2=None, op0=OP.add)
                    # A2A over 8 cores: shard s rows [192s,+192) = head-s q/k/v of my tokens
                    for s_ in range(8):
                        pb = 64 * (s_ % 2)
                        blk = s_ // 2
                        nc.sync.dma_start(out=qkv_ai[l][192 * s_:192 * s_ + 64, :],
                                          in_=qkvT[pb:pb + 64, 512 * blk:512 * (blk + 1)])
                        nc.sync.dma_start(out=qkv_ai[l][192 * s_ + 64:192 * s_ + 128, :],
                                          in_=qkvT[pb:pb + 64, 512 * (4 + blk):512 * (5 + blk)])
                        nc.sync.dma_start(out=qkv_ai[l][192 * s_ + 128:192 * s_ + 192, :],
                                          in_=qkvT[pb:pb + 64, 512 * (8 + blk):512 * (9 + blk)])
                    nc.gpsimd.collective_compute(
                        "AllToAll", OP.bypass, replica_groups=grp,
                        ins=[qkv_ai[l][:]], outs=[qkv_ao[l][:]],
                    )
                    # my head over both batches: batch b in partition half 64b
                    qT = wk.tile([128, 2048], BF16, tag="qT", bufs=1)
                    kT = wk.tile([128, 2048], BF16, tag="kT", bufs=1)
                    vT = wk.tile([128, 2048], BF16, tag="vT", bufs=1)
                    for r in range(8):
                        b_, rr = r // 4, r % 4
                        nc.sync.dma_start(out=qT[64 * b_:64 * b_ + 64, 512 * rr:512 * (rr + 1)],
                                          in_=qkv_ao[l][192 * r:192 * r + 64, :])
                        nc.sync.dma_start(out=kT[64 * b_:64 * b_ + 64, 512 * rr:512 * (rr + 1)],
                                          in_=qkv_ao[l][192 * r + 64:192 * r + 128, :])
                        nc.sync.dma_start(out=vT[64 * b_:64 * b_ + 64, 512 * rr:512 * (rr + 1)],
                                          in_=qkv_ao[l][192 * r + 128:192 * r + 192, :])
                    for b_ in range(2):
                        hb = 64 * b_
                        for i in range(16):
                            tp = pmm.tile([128, 64], BF16, tag="pmm")
                            nc.tensor.transpose(out=tp[:], in_=vT[hb:hb + 64, 128 * i:128 * (i + 1)],
                                                identity=ident[hb:hb + 64, hb:hb + 64])
                            nc.vector.tensor_copy(out=vones[:, 65 * (16 * b_ + i):65 * (16 * b_ + i) + 64],
                                                  in_=tp[:])

                    # -- attention (my head, both batches, causal, q in 1024-pairs) --
                    attnT = wk.tile([64, 4096], BF16, tag="attnT", bufs=1)
                    for b_ in range(2):
                        hb = 64 * b_
                        for p in range(2):
                            outp = pout.tile([65, 1024], F32, tag="pout")
                            for i in range(8 * p + 8):
                                jlmin = max(0, i // 4 - 2 * p)
                                for jl in (0, 1):
                                    if jl < jlmin:
                                        continue
                                    diag = (i // 4 == 2 * p + jl)
                                    sc = psc.tile([128, 512], F32, tag="psc")
                                    nc.tensor.matmul(
                                        sc[:],
                                        lhsT=kT[hb:hb + 64, 128 * i:128 * (i + 1)],
                                        rhs=qT[hb:hb + 64, 1024 * p + 512 * jl:1024 * p + 512 * (jl + 1)],
                                        start=True, stop=not diag)
                                    if diag:
                                        m = i % 4
                                        nc.tensor.matmul(
                                            sc[:], lhsT=ident[:],
                                            rhs=msk[:, 384 - 128 * m:896 - 128 * m],
                                            start=False, stop=True)
                                    ex = ep.tile([128, 512], BF16, tag="ex")
                                    nc.scalar.activation(out=ex[:], in_=sc[:],
                                                         func=AF.Exp, scale=float(HD) ** -0.5)
                                    kmax = 4 * (2 * p + jl) + 3
                                    nc.tensor.matmul(
                                        outp[:, 512 * jl:512 * (jl + 1)],
                                        lhsT=vones[:, 65 * (16 * b_ + i):65 * (16 * b_ + i + 1)],
                                        rhs=ex[:],
                                        start=(i == 0), stop=(i == kmax))
                            # normalize: rows 0..63 /= row 64
                            # (bcast denom via PE, fast-reciprocal on 64 lanes, multiply)
                            dnb = wk.tile([1, 1024], BF16, tag="rcb", bufs=1)
                            nc.vector.tensor_copy(out=dnb[:], in_=outp[64:65, :])
                            for q2 in range(2):
                                bc = psc.tile([64, 512], F32, tag="psc")
                                nc.tensor.matmul(bc[:], lhsT=ones_r[:, 0:64],
                                                 rhs=dnb[:, 512 * q2:512 * (q2 + 1)], start=True, stop=True)
                                rcs = wk.tile([64, 512], F32, tag="bcs", bufs=1)
                                nc.vector.reciprocal_approx_fast(out=rcs[:], in_=bc[:])
                                nc.vector.tensor_tensor(
                                    out=attnT[:, 2048 * b_ + 1024 * p + 512 * q2:2048 * b_ + 1024 * p + 512 * (q2 + 1)],
                                    in0=outp[0:64, 512 * q2:512 * (q2 + 1)], in1=rcs[:], op=OP.mult)
                    # A2A attention outputs back to token owners
                    for s_ in range(8):
                        nc.sync.dma_start(out=att_ai[l][64 * s_:64 * (s_ + 1), :],
                                          in_=attnT[:, 512 * s_:512 * (s_ + 1)])
                    nc.gpsimd.collective_compute(
                        "AllToAll", OP.bypass, replica_groups=grp,
                        ins=[att_ai[l][:]], outs=[att_ao[l][:]],
                    )
                    aT = wk.tile([128, 4 * 512], BF16, tag="aT", bufs=1)
                    for r in range(4):
                        nc.sync.dma_start(out=aT[:, 512 * r:512 * (r + 1)],
                                          in_=att_ao[l][128 * r:128 * (r + 1), :])

                    # -- proj + residual --
                    for t in range(4):
                        ps = pmm.tile([128, 512], F32, tag="pmm")
                        for fc in range(4):
                            nc.tensor.matmul(ps[:],
                                             lhsT=aT[:, 512 * fc + 128 * t:512 * fc + 128 * (t + 1)],
                                             rhs=wpj[:, 512 * fc:512 * (fc + 1)],
                                             start=(fc == 0), stop=(fc == 3))
                        nc.vector.tensor_tensor(out=ps[:], in0=ps[:],
                                                in1=projb_bc[:, D * l:D * (l + 1)], op=OP.add)
                        nc.vector.tensor_tensor(out=h[:, D * t:D * (t + 1)],
                                                in0=h[:, D * t:D * (t + 1)], in1=ps[:], op=OP.add)

                    # -- LN2 + FFN --
                    hln2T = wk.tile([128, 4 * 512], BF16, tag="hlnT", bufs=1)
                    layernorm_T(h[:], hln2T)
                    fT = wk.tile([128, 16 * 512], BF16, tag="fT", bufs=1)
                    for o in range(16):
                        ps = pmm.tile([128, 512], F32, tag="pmm")
                        for kc in range(4):
                            nc.tensor.matmul(ps[:],
                                             lhsT=wf1[:, 2048 * kc + 128 * o:2048 * kc + 128 * (o + 1)],
                                             rhs=hln2T[:, 512 * kc:512 * (kc + 1)],
                                             start=(kc == 0), stop=(kc == 3))
                        nc.scalar.activation(out=fT[:, 512 * o:512 * (o + 1)], in_=ps[:],
                                             func=AF.Gelu, bias=bf1[:, o:o + 1])
                    for t in range(4):
                        ps = pmm.tile([128, 512], F32, tag="pmm")
                        for kc in range(16):
                            nc.tensor.matmul(ps[:],
                                             lhsT=fT[:, 512 * kc + 128 * t:512 * kc + 128 * (t + 1)],
                                             rhs=wf2[:, 512 * kc:512 * (kc + 1)],
                                             start=(kc == 0), stop=(kc == 15))
                        nc.vector.tensor_tensor(out=ps[:], in0=ps[:],
                                                in1=ffn2b_bc[:, D * l:D * (l + 1)], op=OP.add)
                        nc.vector.tensor_tensor(out=h[:, D * t:D * (t + 1)],
                                                in0=h[:, D * t:D * (t + 1)], in1=ps[:], op=OP.add)

                # ================= final LN + AllGather =================
                hfT = wk.tile([128, 4 * 512], BF16, tag="hlnT", bufs=1)
                layernorm_T(h[:], hfT)
                for f in range(4):
                    nc.sync.dma_start(out=ag_in[128 * f:128 * (f + 1), :],
                                      in_=hfT[:, 512 * f:512 * (f + 1)])
                nc.gpsimd.collective_compute(
                    "AllGather", OP.bypass, replica_groups=[list(range(NC))],
                    ins=[ag_in[:]], outs=[ag_out[:]],
                )

            # ================= LM head (body pools closed, PSUM free) ========
            with (
                tc.tile_pool(name="lmw", bufs=1) as lw,
                tc.tile_pool(name="lmwork", bufs=3) as lk,
                tc.tile_pool(name="plm", bufs=2, space="PSUM") as plm,
            ):
                lmw = lw.tile([128, 4 * VSH], BF16, name="lmw")
                nc.sync.dma_start(out=lmw[:].rearrange("p (c e) -> p c e", c=4),
                                  in_=lmT[:].rearrange("(c p) e -> p c e", p=128))
                for tt in range(32):
                    r, u = tt // 4, tt % 4
                    lhs = lk.tile([128, 4 * 128], BF16, tag="lhs")
                    for kc in range(4):
                        nc.sync.dma_start(out=lhs[:, 128 * kc:128 * (kc + 1)],
                                          in_=ag_out[512 * r + 128 * kc:512 * r + 128 * (kc + 1),
                                                     128 * u:128 * (u + 1)])
                    stage = lk.tile([128, VSH], F32, tag="stage")
                    for hf in range(2):
                        ps = plm.tile([128, 2048], F32, tag="plm")
                        for kc in range(4):
                            for vc in range(4):
                                w0 = VSH * kc + 2000 * hf + 500 * vc
                                nc.tensor.matmul(
                                    ps[:, 512 * vc:512 * vc + 500],
                                    lhsT=lhs[:, 128 * kc:128 * (kc + 1)],
                                    rhs=lmw[:, w0:w0 + 500],
                                    start=(kc == 0), stop=(kc == 3))
                        ps3 = ps[:].rearrange("p (b e) -> p b e", b=4)[:, :, 0:500]
                        st3 = stage[:, 2000 * hf:2000 * (hf + 1)].rearrange("p (b e) -> p b e", b=4)
                        bc3 = lmb_bc[:, 2000 * hf:2000 * (hf + 1)].rearrange("p (b e) -> p b e", b=4)
                        nc.vector.tensor_tensor(out=st3, in0=ps3, in1=bc3, op=OP.add)
                    nc.sync.dma_start(out=logits[128 * tt:128 * (tt + 1), :], in_=stage[:])

    nc.compile()
    return nc


_NC_CACHE = None


def _get_nc():
    global _NC_CACHE
    if _NC_CACHE is None:
        _NC_CACHE = build_nc()
    return _NC_CACHE


def _prep_inputs(inputs):
    bf = ml_dtypes.bfloat16
    tok_emb = np.asarray(inputs["tok_emb"], np.float32)
    pos_emb = np.asarray(inputs["pos_emb"], np.float32)
    x = np.asarray(inputs["x"]).astype(np.int32).reshape(-1)  # [4096] flat

    def eff(w, g, b, wb):
        # fold the preceding layernorm's gamma/beta into w (out,in) and bias
        w = np.asarray(w, np.float32)
        weff = w * np.asarray(g, np.float32)[None, :]
        beff = w @ np.asarray(b, np.float32) + np.asarray(wb, np.float32)
        return weff, beff

    wqkvT = np.zeros((L, D, 3 * D), bf)
    bqkv = np.zeros((L, 12, 128), np.float32)
    wprojT = np.zeros((L, D, D), bf)
    bproj = np.zeros((L, 1, D), np.float32)
    wffn1T = np.zeros((L, D, DFF), bf)
    bffn1 = np.zeros((L, 16, 128), np.float32)
    wffn2T = np.zeros((L, DFF, D), bf)
    bffn2 = np.zeros((L, 1, D), np.float32)
    for l in range(L):
        w, b = eff(inputs["qkv_w"][l], inputs["ln1_g"][l], inputs["ln1_b"][l], inputs["qkv_b"][l])
        wqkvT[l] = w.T.astype(bf)
        bqkv[l] = b.reshape(12, 128)
        wprojT[l] = np.asarray(inputs["proj_w"][l], np.float32).T.astype(bf)
        bproj[l, 0] = np.asarray(inputs["proj_b"][l], np.float32)
        w, b = eff(inputs["ffn1_w"][l], inputs["ln2_g"][l], inputs["ln2_b"][l], inputs["ffn1_b"][l])
        wffn1T[l] = w.T.astype(bf)
        bffn1[l] = b.reshape(16, 128)
        wffn2T[l] = np.asarray(inputs["ffn2_w"][l], np.float32).T.astype(bf)
        bffn2[l, 0] = np.asarray(inputs["ffn2_b"][l], np.float32)
    lmw, lmbf = eff(inputs["lm_w"], inputs["lnf_g"], inputs["lnf_b"], inputs["lm_b"])

    mstrip = np.full((128, 896), NEG, np.float32)
    kk = np.arange(128)[:, None]
    cc = np.arange(896)[None, :]
    mstrip[kk <= cc - 384] = 0.0
    mstrip = mstrip.astype(bf)

    common = dict(tok_emb=tok_emb, wqkvT=wqkvT, bqkv=bqkv.reshape(L * 12 * 128, 1),
                  wprojT=wprojT, bproj=bproj, wffn1T=wffn1T,
                  bffn1=bffn1.reshape(L * 16 * 128, 1), wffn2T=wffn2T, bffn2=bffn2,
                  mstrip=mstrip, ident_in=np.eye(128, dtype=bf),
                  ones_in=np.ones((1, 128), bf))
    in_maps = []
    for c in range(NC):
        s0 = 512 * (c % 4)
        m = dict(common)
        m["pos"] = pos_emb[s0:s0 + 512]
        m["xidx"] = x[512 * c:512 * (c + 1)].reshape(TOK, 1)
        m["lmT"] = np.ascontiguousarray(lmw[VSH * c:VSH * (c + 1)].T.astype(bf))
        m["lmb"] = lmbf[VSH * c:VSH * (c + 1)].reshape(1, VSH).copy()
        in_maps.append(m)
    return in_maps


def run(inputs, trace=False, tmpdir=None):
    nc = _get_nc()
    in_maps = _prep_inputs(inputs)
    res = bass_utils.run_bass_kernel_spmd(nc, in_maps, list(range(NC)), trace=trace, tmpdir=tmpdir)
    full = np.empty((B * S, V), np.float32)
    for c in range(NC):
        full[:, VSH * c:VSH * (c + 1)] = res.results[c]["logits"]
    return full.reshape(B, S, V), res


def kernel(**inputs) -> np.ndarray:
    out, _ = run(inputs)
    return out
